# revision 1
# baseline (speedup 1.0000x reference)
"""Bidirectional Mamba layer on 8 Trainium2 NeuronCores.

Sharding: data-parallel over batch (8 batches -> 8 cores). Each core runs
both directions (fwd on x, bwd on time-reversed x) for its batch.

Per-core algorithm (per direction), all in "d-major" layout [d on
partitions, time on free dim]:
  1. uzT = in_w @ x^T                (PE, bf16)
  2. causal depthwise conv + SiLU    (ACT scale-copy + 3 fused DVE STT)
  3. dblT = xp_w @ uc^T              (PE)  -> dt / B / C rows
  4. deltaT = softplus(dt_w @ dtT + dt_b)  (PE + ACT Softplus)
  5. per (d-chunk, s):  a = exp(A[d,s] * delta)   (ACT, per-partition scale)
                        b = (delta*uc) * bcast(B[s,:])  (DVE)
                        h = tensor_tensor_scan(a, b)    (DVE, fp32 state)
                        y += h * bcast(C[s,:])          (DVE)
  6. g = (uc*D + y) * silu(z); out = g^T @ out_w^T      (PE)
Host combines: out = out_f + reverse_time(out_b).
"""

import sys

sys.path.insert(0, "/opt/trn_rl_repo")

import numpy as np
import ml_dtypes

import concourse.bass as bass
import concourse.mybir as mybir
import bass_rust
from concourse import tile
from concourse.bass_utils import run_bass_kernel_spmd

BF16 = mybir.dt.bfloat16
F32 = mybir.dt.float32
AF = mybir.ActivationFunctionType
OP = mybir.AluOpType

D_MODEL = 512
D_INNER = 1024
D_STATE = 16
D_CONV = 4
DT_RANK = 32
BATCH = 8
SEQ = 1024

P = 128
NC_D = D_INNER // P  # 8 d-chunks
NC_T = SEQ // P      # 8 t-chunks
NN = SEQ // 512      # 2 psum-free chunks


def _dir_params(nc, d):
    """Declare per-direction dram parameters (host passes pre-transposed)."""
    return {
        "inwT": nc.declare_dram_parameter(f"inwT_{d}", [D_MODEL, 2 * D_INNER], BF16, isOutput=False),
        "xpwT": nc.declare_dram_parameter(f"xpwT_{d}", [D_INNER, DT_RANK + 2 * D_STATE], BF16, isOutput=False),
        "dtwT": nc.declare_dram_parameter(f"dtwT_{d}", [DT_RANK, D_INNER], BF16, isOutput=False),
        "outwT": nc.declare_dram_parameter(f"outwT_{d}", [D_INNER, D_MODEL], BF16, isOutput=False),
        "A": nc.declare_dram_parameter(f"A_{d}", [D_INNER, D_STATE], F32, isOutput=False),
        "convw": nc.declare_dram_parameter(f"convw_{d}", [D_INNER, D_CONV], F32, isOutput=False),
        "convb": nc.declare_dram_parameter(f"convb_{d}", [D_INNER, 1], F32, isOutput=False),
        "dtb": nc.declare_dram_parameter(f"dtb_{d}", [D_INNER, 1], F32, isOutput=False),
        "Dp": nc.declare_dram_parameter(f"Dp_{d}", [D_INNER, 1], F32, isOutput=False),
        "xT": nc.declare_dram_parameter(f"xT_{d}", [D_MODEL, SEQ], BF16, isOutput=False),
        "out": nc.declare_dram_parameter(f"out_{d}", [SEQ, D_MODEL], F32, isOutput=True),
        "oht": nc.declare_dram_parameter(f"oht_{d}", [2 * D_STATE, 2 * D_STATE * P], BF16, isOutput=False),
    }


def _one_direction(ctx_pools, tc, p):
    import contextlib

    nc = tc.nc

    cst = ctx_pools  # long-lived pool for this direction

    # ---- load weights ----
    inwT = [cst.tile([P, 2 * D_INNER], BF16, tag=f"inwT{k}", name=f"inwT{k}") for k in range(4)]
    for k in range(4):
        nc.sync.dma_start(inwT[k][:], p["inwT"][k * P:(k + 1) * P, :])
    xT = [cst.tile([P, SEQ], BF16, tag=f"xT{k}", name=f"xT{k}") for k in range(4)]
    for k in range(4):
        nc.sync.dma_start(xT[k][:], p["xT"][k * P:(k + 1) * P, :])
    xpwT = [cst.tile([P, 64], BF16, tag=f"xpwT{c}", name=f"xpwT{c}") for c in range(NC_D)]
    outwT = [cst.tile([P, D_MODEL], BF16, tag=f"outwT{c}", name=f"outwT{c}") for c in range(NC_D)]
    A_sb = [cst.tile([P, D_STATE], F32, tag=f"A{c}", name=f"A{c}") for c in range(NC_D)]
    convw = [cst.tile([P, D_CONV], F32, tag=f"convw{c}", name=f"convw{c}") for c in range(NC_D)]
    convb = [cst.tile([P, 1], F32, tag=f"convb{c}", name=f"convb{c}") for c in range(NC_D)]
    dtb = [cst.tile([P, 1], F32, tag=f"dtb{c}", name=f"dtb{c}") for c in range(NC_D)]
    Dp = [cst.tile([P, 1], F32, tag=f"Dp{c}", name=f"Dp{c}") for c in range(NC_D)]
    for c in range(NC_D):
        sl = slice(c * P, (c + 1) * P)
        nc.sync.dma_start(xpwT[c][:], p["xpwT"][sl, :])
        nc.sync.dma_start(outwT[c][:], p["outwT"][sl, :])
        nc.sync.dma_start(A_sb[c][:], p["A"][sl, :])
        nc.sync.dma_start(convw[c][:], p["convw"][sl, :])
        nc.sync.dma_start(convb[c][:], p["convb"][sl, :])
        nc.sync.dma_start(dtb[c][:], p["dtb"][sl, :])
        nc.sync.dma_start(Dp[c][:], p["Dp"][sl, :])
    dtwT = cst.tile([DT_RANK, D_INNER], BF16, tag="dtwT", name="dtwT")
    nc.sync.dma_start(dtwT[:], p["dtwT"][:])

    # persistent activations for this direction
    uT = [cst.tile([P, SEQ + D_CONV - 1], BF16, tag=f"uT{c}", name=f"uT{c}") for c in range(NC_D)]
    sz = [cst.tile([P, SEQ], BF16, tag=f"sz{c}", name=f"sz{c}") for c in range(NC_D)]
    ucT = [cst.tile([P, SEQ], BF16, tag=f"ucT{c}", name=f"ucT{c}") for c in range(NC_D)]
    delta = [cst.tile([P, SEQ], BF16, tag=f"delta{c}", name=f"delta{c}") for c in range(NC_D)]
    w_bf = [cst.tile([P, SEQ], BF16, tag=f"w{c}", name=f"w{c}") for c in range(NC_D)]
    y_sb = [cst.tile([P, SEQ], BF16, tag=f"y{c}", name=f"y{c}") for c in range(NC_D)]
    dt_bf = cst.tile([DT_RANK, SEQ], BF16, tag="dt_bf", name="dt_bf")
    bc_bf = cst.tile([2 * D_STATE, SEQ], BF16, tag="bc_bf", name="bc_bf")

    for c in range(NC_D):
        nc.vector.memset(uT[c][:, 0:D_CONV - 1], 0.0)

    with contextlib.ExitStack() as phase:
        ps1 = phase.enter_context(tc.tile_pool(name="ps1", bufs=4, space="PSUM"))
        # ---- GEMM1: uzT[m*128:(m+1)*128, :] ----
        for m in range(2 * NC_D):
            for n in range(NN):
                pt = ps1.tile([P, 512], F32, tag="g1", name="g1")
                for k in range(4):
                    nc.tensor.matmul(
                        pt[:],
                        inwT[k][:, m * P:(m + 1) * P],
                        xT[k][:, n * 512:(n + 1) * 512],
                        start=(k == 0),
                        stop=(k == 3),
                    )
                if m < NC_D:
                    nc.scalar.copy(
                        uT[m][:, D_CONV - 1 + n * 512: D_CONV - 1 + (n + 1) * 512],
                        pt[:],
                    )
                else:
                    nc.scalar.activation(
                        sz[m - NC_D][:, n * 512:(n + 1) * 512], pt[:], AF.Silu
                    )

        # ---- conv + SiLU ----
        t_pool = phase.enter_context(tc.tile_pool(name="conv_t", bufs=2))
        for c in range(NC_D):
            taps = []
            for k in range(D_CONV):
                tk = t_pool.tile([P, SEQ], BF16, tag="tk", name="tk", bufs=5)
                nc.scalar.activation(
                    tk[:], uT[c][:, k:k + SEQ], AF.Copy, scale=convw[c][:, k:k + 1]
                )
                taps.append(tk)
            s01 = t_pool.tile([P, SEQ], BF16, tag="s01", name="s01")
            nc.vector.tensor_add(s01[:], taps[0][:], taps[1][:])
            s23 = t_pool.tile([P, SEQ], BF16, tag="s23", name="s23")
            nc.vector.tensor_add(s23[:], taps[2][:], taps[3][:])
            s03 = t_pool.tile([P, SEQ], BF16, tag="s03", name="s03")
            nc.vector.tensor_add(s03[:], s01[:], s23[:])
            nc.scalar.activation(
                ucT[c][:], s03[:], AF.Silu, bias=convb[c][:, 0:1]
            )

    with contextlib.ExitStack() as phase:
        ps2 = phase.enter_context(tc.tile_pool(name="ps2", bufs=4, space="PSUM"))
        # ---- GEMM2: dblT [64, SEQ] ----
        for n in range(NN):
            pt = ps2.tile([64, 512], F32, tag="g2", name="g2")
            for c in range(NC_D):
                nc.tensor.matmul(
                    pt[:], xpwT[c][:], ucT[c][:, n * 512:(n + 1) * 512],
                    start=(c == 0), stop=(c == NC_D - 1),
                )
            nc.vector.tensor_copy(dt_bf[:, n * 512:(n + 1) * 512], pt[0:DT_RANK, :])
            nc.vector.tensor_copy(
                bc_bf[:, n * 512:(n + 1) * 512], pt[DT_RANK:64, :]
            )

        # ---- GEMM3: deltaT = softplus(dt_w @ dtT + dt_b) ----
        # softplus(x) = relu(x) + ln(1 + exp(-|x|))  (Softplus has no ACT table set)
        t_pool2 = phase.enter_context(tc.tile_pool(name="sp_t", bufs=3))
        for m in range(NC_D):
            for n in range(NN):
                pt = ps2.tile([P, 512], F32, tag="g3", name="g3")
                nc.tensor.matmul(
                    pt[:], dtwT[:, m * P:(m + 1) * P],
                    dt_bf[:, n * 512:(n + 1) * 512],
                    start=True, stop=True,
                )
                sl = slice(n * 512, (n + 1) * 512)
                ab = t_pool2.tile([P, 512], F32, tag="sp_ab", name="sp_ab")
                nc.scalar.activation(ab[:], pt[:], AF.Abs, bias=dtb[m][:, 0:1])
                en = t_pool2.tile([P, 512], F32, tag="sp_en", name="sp_en")
                nc.scalar.activation(en[:], ab[:], AF.Exp, scale=-1.0)
                l1 = t_pool2.tile([P, 512], F32, tag="sp_l1", name="sp_l1")
                nc.scalar.activation(l1[:], en[:], AF.Ln, bias=1.0)
                rl = t_pool2.tile([P, 512], F32, tag="sp_rl", name="sp_rl")
                nc.scalar.activation(rl[:], pt[:], AF.Relu, bias=dtb[m][:, 0:1])
                nc.vector.tensor_add(delta[m][:, sl], rl[:], l1[:])

        # ---- w = delta * uc ----
        for c in range(NC_D):
            nc.vector.tensor_mul(w_bf[c][:], delta[c][:], ucT[c][:])

    # ---- scan phase ----
    oht = cst.tile([2 * D_STATE, 2 * D_STATE * P], BF16, tag="oht", name="oht")
    nc.sync.dma_start(oht[:], p["oht"][:])
    with contextlib.ExitStack() as phase:
        bcp_pool = phase.enter_context(tc.tile_pool(name="bcp", bufs=4, space="PSUM"))
        bc_pool = phase.enter_context(tc.tile_pool(name="bc", bufs=3))
        ab_pool = phase.enter_context(tc.tile_pool(name="ab", bufs=4))
        h_pool = phase.enter_context(tc.tile_pool(name="h", bufs=3))
        for s in range(D_STATE):
            Bbc = bc_pool.tile([P, SEQ], BF16, tag="Bbc", name="Bbc")
            Cbc = bc_pool.tile([P, SEQ], BF16, tag="Cbc", name="Cbc")
            for src_row, dst in ((s, Bbc), (D_STATE + s, Cbc)):
                ps = bcp_pool.tile([P, SEQ], F32, tag="bcps", name="bcps")
                for n in range(NN):
                    nc.tensor.matmul(
                        ps[:, n * 512:(n + 1) * 512],
                        oht[:, src_row * P:(src_row + 1) * P],
                        bc_bf[:, n * 512:(n + 1) * 512],
                        start=True,
                        stop=True,
                    )
                nc.scalar.copy(dst[:], ps[:])
            for c in range(NC_D):
                a_t = ab_pool.tile([P, SEQ], BF16, tag="a", name="a")
                nc.scalar.activation(
                    a_t[:], delta[c][:], AF.Exp, scale=A_sb[c][:, s:s + 1]
                )
                b_t = ab_pool.tile([P, SEQ], BF16, tag="b", name="b")
                nc.vector.tensor_mul(b_t[:], w_bf[c][:], Bbc[:])
                h_t = h_pool.tile([P, SEQ], BF16, tag="h", name="h")
                nc.vector.tensor_tensor_scan(
                    h_t[:], a_t[:], b_t[:], 0.0, op0=OP.mult, op1=OP.add
                )
                if s == 0:
                    nc.vector.tensor_mul(y_sb[c][:], h_t[:], Cbc[:])
                else:
                    t_t = h_pool.tile([P, SEQ], BF16, tag="yt", name="yt")
                    nc.vector.tensor_mul(t_t[:], h_t[:], Cbc[:])
                    nc.vector.tensor_add(y_sb[c][:], y_sb[c][:], t_t[:])

    # ---- gate: y = (uc*D + y) * silu(z) ----
    for c in range(NC_D):
        ucd = cst.tile([P, SEQ], BF16, tag=f"ucd{c}", name=f"ucd{c}")
        nc.scalar.activation(ucd[:], ucT[c][:], AF.Copy, scale=Dp[c][:, 0:1])
        nc.vector.tensor_add(y_sb[c][:], y_sb[c][:], ucd[:])
        nc.vector.tensor_mul(y_sb[c][:], y_sb[c][:], sz[c][:])

    # ---- GEMM4: out[m*128:(m+1)*128, :] = g^T @ out_w^T ----
    with contextlib.ExitStack() as phase:
        ps4 = phase.enter_context(tc.tile_pool(name="ps4", bufs=3, space="PSUM"))
        o_pool = phase.enter_context(tc.tile_pool(name="o", bufs=3))
        for m in range(NC_T):
            pt = ps4.tile([P, D_MODEL], F32, tag="g4", name="g4")
            for c in range(NC_D):
                nc.tensor.matmul(
                    pt[:], y_sb[c][:, m * P:(m + 1) * P], outwT[c][:],
                    start=(c == 0), stop=(c == NC_D - 1),
                )
            ot = o_pool.tile([P, D_MODEL], F32, tag="ot", name="ot")
            nc.vector.tensor_copy(ot[:], pt[:])
            nc.sync.dma_start(p["out"][m * P:(m + 1) * P, :], ot[:])


def _split_excess_waits(nc):
    """walrus in this toolchain accepts at most one sync-wait per
    instruction (two for EventSemaphore); hoist the excess onto injected
    same-engine NoOps placed directly before the instruction."""
    for f in nc.m.functions:
        for bb in f.blocks:
            new_insts = []
            for inst in bb.instructions:
                si = inst.sync_info
                cap = 2 if isinstance(inst, mybir.InstEventSemaphore) else 1
                if si is not None and len(si.on_wait) > cap:
                    waits = list(si.on_wait)
                    for i, w in enumerate(waits[:-cap]):
                        nop = mybir.InstNoOp(
                            name=f"{inst.name}-wsplit{i}", ins=[], outs=[]
                        )
                        nop.engine = inst.engine
                        nop.sync_info = bass_rust.SyncInfo(on_wait=[w], on_update=[])
                        new_insts.append(nop)
                    inst.sync_info = bass_rust.SyncInfo(
                        on_wait=waits[-cap:], on_update=list(si.on_update)
                    )
                new_insts.append(inst)
            try:
                bb.instructions = new_insts
            except Exception:
                bb.instructions.clear()
                bb.instructions.extend(new_insts)


def build_bass():
    nc = bass.Bass()
    params = {d: _dir_params(nc, d) for d in ("f", "b")}
    with tile.TileContext(nc) as tc:
        for d in ("f", "b"):
            with tc.tile_pool(name=f"cst_{d}", bufs=1) as cst:
                _one_direction(cst, tc, params[d])
    _split_excess_waits(nc)
    return nc


def _prep_dir(w):
    """Host-side prep of one direction's weights -> dram param arrays."""
    bf = ml_dtypes.bfloat16
    in_w, conv_w, conv_b, xp_w, dt_w, dt_b, A_log, Dp, out_w = w
    return {
        "inwT": np.ascontiguousarray(in_w.T).astype(bf),
        "xpwT": np.ascontiguousarray(xp_w.T).astype(bf),
        "dtwT": np.ascontiguousarray(dt_w.T).astype(bf),
        "outwT": np.ascontiguousarray(out_w.T).astype(bf),
        "A": np.ascontiguousarray(-np.exp(A_log.astype(np.float64))).astype(np.float32),
        "convw": np.ascontiguousarray(conv_w).astype(np.float32),
        "convb": np.ascontiguousarray(conv_b).reshape(D_INNER, 1).astype(np.float32),
        "dtb": np.ascontiguousarray(dt_b).reshape(D_INNER, 1).astype(np.float32),
        "Dp": np.ascontiguousarray(Dp).reshape(D_INNER, 1).astype(np.float32),
        "oht": np.kron(np.eye(2 * D_STATE, dtype=np.float32), np.ones((1, P), np.float32)).astype(bf),
    }


_CACHED = {}


def kernel(
    x,
    in_w_f, conv_w_f, conv_b_f, xp_w_f, dt_w_f, dt_b_f, A_log_f, D_f, out_w_f,
    in_w_b, conv_w_b, conv_b_b, xp_w_b, dt_w_b, dt_b_b, A_log_b, D_b, out_w_b,
):
    bf = ml_dtypes.bfloat16
    x = np.asarray(x, dtype=np.float32)

    if "nc" not in _CACHED:
        _CACHED["nc"] = build_bass()
    nc = _CACHED["nc"]

    wf = _prep_dir((in_w_f, conv_w_f, conv_b_f, xp_w_f, dt_w_f, dt_b_f,
                    A_log_f, D_f, out_w_f))
    wb = _prep_dir((in_w_b, conv_w_b, conv_b_b, xp_w_b, dt_w_b, dt_b_b,
                    A_log_b, D_b, out_w_b))

    in_maps = []
    for b in range(BATCH):
        m = {}
        for d, wd in (("f", wf), ("b", wb)):
            for k, v in wd.items():
                m[f"{k}_{d}"] = v
        m["xT_f"] = np.ascontiguousarray(x[b].T).astype(bf)
        m["xT_b"] = np.ascontiguousarray(x[b][::-1].T).astype(bf)
        in_maps.append(m)

    res = run_bass_kernel_spmd(nc, in_maps, core_ids=list(range(BATCH)))
    out = np.empty((BATCH, SEQ, D_MODEL), np.float32)
    for b in range(BATCH):
        rb = res.results[b]
        out[b] = rb["out_f"] + rb["out_b"][::-1]
    return out



# revision 5
# speedup vs baseline: 1.5470x; 1.5470x over previous
"""Bidirectional Mamba layer on 8 Trainium2 NeuronCores.

Sharding: data-parallel over batch (8 batches -> 8 cores). Each core runs
both directions (fwd on x, bwd on time-reversed x) for its batch.

v2: engine-rebalanced vs the v1 baseline.
  - depthwise conv on PE (diag-block matmuls, PSUM tap accumulation)
  - all PSUM evacuations + the y-accumulation on the Pool engine
  - softplus as Exp+Ln (same ACT table as the 256 scan exps -> no
    activation-table reloads in the scan phase)
  - y initialized to uc*D in the prelude (drops the gate add, frees ucT)
  - z parked in scratch DRAM, reloaded + silu'd at gate time (SBUF diet)
  - big per-direction arrays (delta/w/y) live in tag-rotated pools so the
    two directions share SBUF; dir-b's prelude overlaps dir-f's scan tail
"""

import sys

sys.path.insert(0, "/opt/trn_rl_repo")

import numpy as np
import ml_dtypes

import concourse.bass as bass
import concourse.mybir as mybir
import bass_rust
from concourse import tile
from concourse.bass_utils import run_bass_kernel_spmd

BF16 = mybir.dt.bfloat16
F32 = mybir.dt.float32
AF = mybir.ActivationFunctionType
OP = mybir.AluOpType

D_MODEL = 512
D_INNER = 1024
D_STATE = 16
D_CONV = 4
DT_RANK = 32
BATCH = 8
SEQ = 1024

P = 128
NC_D = D_INNER // P  # 8 d-chunks
NN = SEQ // 512      # 2 psum-free chunks

# y-accumulate adds routed to DVE for these state indices (rest on Pool)
YADD_DVE_S = frozenset((13, 14, 15))


def _dir_params(nc, d):
    ps = {
        "inwT": nc.declare_dram_parameter(f"inwT_{d}", [D_MODEL, 2 * D_INNER], BF16, isOutput=False),
        "xpwT": nc.declare_dram_parameter(f"xpwT_{d}", [D_INNER, DT_RANK + 2 * D_STATE], BF16, isOutput=False),
        "dtwT": nc.declare_dram_parameter(f"dtwT_{d}", [DT_RANK, D_INNER], BF16, isOutput=False),
        "outwT": nc.declare_dram_parameter(f"outwT_{d}", [D_INNER, D_MODEL], BF16, isOutput=False),
        "A": nc.declare_dram_parameter(f"A_{d}", [D_INNER, D_STATE], F32, isOutput=False),
        "convdiag": nc.declare_dram_parameter(f"convdiag_{d}", [D_CONV * P, D_INNER], BF16, isOutput=False),
        "convb": nc.declare_dram_parameter(f"convb_{d}", [D_INNER, 1], F32, isOutput=False),
        "dtb": nc.declare_dram_parameter(f"dtb_{d}", [D_INNER, 1], F32, isOutput=False),
        "Dp": nc.declare_dram_parameter(f"Dp_{d}", [D_INNER, 1], F32, isOutput=False),
        "xT": nc.declare_dram_parameter(f"xT_{d}", [D_MODEL, SEQ], BF16, isOutput=False),
        "out": nc.declare_dram_parameter(f"out_{d}", [SEQ, D_MODEL], F32, isOutput=True),
    }
    ps["zscr"] = nc.dram_tensor(f"zscr_{d}", [D_INNER, SEQ], BF16)
    return ps


def _prelude(tc, pools, p, d, oht):
    """GEMM1 + conv + GEMM2 + softplus + w/y init for one direction."""
    nc = tc.nc
    cst, trans, psp, big = pools["cst"], pools["trans"], pools["psum"], pools["big"]

    # ---- weights ----
    inwT = [trans.tile([P, 2 * D_INNER], BF16, tag="inwT", name=f"inwT{d}{k}", bufs=4) for k in range(4)]
    xT = [trans.tile([P, SEQ], BF16, tag="xT", name=f"xT{d}{k}", bufs=4) for k in range(4)]
    for k in range(4):
        nc.sync.dma_start(inwT[k][:], p["inwT"][k * P:(k + 1) * P, :])
        nc.sync.dma_start(xT[k][:], p["xT"][k * P:(k + 1) * P, :])
    convdiag = [trans.tile([P, D_INNER], BF16, tag="cvd", name=f"cvd{d}{k}", bufs=4) for k in range(D_CONV)]
    for k in range(D_CONV):
        nc.sync.dma_start(convdiag[k][:], p["convdiag"][k * P:(k + 1) * P, :])
    xpwT = [cst.tile([P, 64], BF16, tag=f"xpwT{d}{c}", name=f"xpwT{d}{c}") for c in range(NC_D)]
    A_sb = [cst.tile([P, D_STATE], F32, tag=f"A{d}{c}", name=f"A{d}{c}") for c in range(NC_D)]
    convb = [cst.tile([P, 1], F32, tag=f"convb{d}{c}", name=f"convb{d}{c}") for c in range(NC_D)]
    dtb = [cst.tile([P, 1], F32, tag=f"dtb{d}{c}", name=f"dtb{d}{c}") for c in range(NC_D)]
    Dp = [cst.tile([P, 1], F32, tag=f"Dp{d}{c}", name=f"Dp{d}{c}") for c in range(NC_D)]
    for c in range(NC_D):
        sl = slice(c * P, (c + 1) * P)
        nc.sync.dma_start(xpwT[c][:], p["xpwT"][sl, :])
        nc.sync.dma_start(A_sb[c][:], p["A"][sl, :])
        nc.sync.dma_start(convb[c][:], p["convb"][sl, :])
        nc.sync.dma_start(dtb[c][:], p["dtb"][sl, :])
        nc.sync.dma_start(Dp[c][:], p["Dp"][sl, :])
    dtwT = cst.tile([DT_RANK, D_INNER], BF16, tag=f"dtwT{d}", name=f"dtwT{d}")
    nc.sync.dma_start(dtwT[:], p["dtwT"][:])

    # rotated big arrays (shared between directions via tag rotation)
    delta = [big.tile([P, SEQ], BF16, tag="delta", name=f"delta{d}{c}", bufs=10) for c in range(NC_D)]
    w_bf = [big.tile([P, SEQ], BF16, tag="w", name=f"w{d}{c}", bufs=10) for c in range(NC_D)]
    y_sb = [big.tile([P, SEQ], BF16, tag="y", name=f"y{d}{c}", bufs=10) for c in range(NC_D)]
    bc_bf = cst.tile([2 * D_STATE, SEQ], BF16, tag=f"bc_bf{d}", name=f"bc_bf{d}")
    dt_bf = trans.tile([DT_RANK, SEQ], BF16, tag="dt_bf", name=f"dt_bf{d}", bufs=2)

    uT = [trans.tile([P, SEQ + D_CONV - 1], BF16, tag="uT", name=f"uT{d}{c}", bufs=4) for c in range(NC_D)]
    ucT = [trans.tile([P, SEQ], BF16, tag="ucT", name=f"ucT{d}{c}", bufs=8) for c in range(NC_D)]
    for c in range(NC_D):
        nc.vector.memset(uT[c][:, 0:D_CONV - 1], 0.0)

    # ---- GEMM1: uzT = in_w @ x^T ----
    for m in range(2 * NC_D):
        for n in range(NN):
            pt = psp.tile([P, 512], F32, tag="g1", name="g1", bufs=2)
            for k in range(4):
                nc.tensor.matmul(
                    pt[:], inwT[k][:, m * P:(m + 1) * P],
                    xT[k][:, n * 512:(n + 1) * 512],
                    start=(k == 0), stop=(k == 3),
                )
            if m < NC_D:
                nc.scalar.copy(
                    uT[m][:, D_CONV - 1 + n * 512: D_CONV - 1 + (n + 1) * 512], pt[:]
                )
            else:
                zst = pools["sp"].tile([P, 512], BF16, tag="zst", name="zst", bufs=3)
                nc.scalar.activation(zst[:], pt[:], AF.Silu)
                nc.sync.dma_start(
                    p["zscr"][(m - NC_D) * P:(m - NC_D + 1) * P, n * 512:(n + 1) * 512],
                    zst[:],
                )

    # ---- conv on PE: uc = silu(sum_k diag(w_k) @ u_shift_k + b) ----
    for c in range(NC_D):
        for n in range(NN):
            pt = psp.tile([P, 512], F32, tag="cv", name="cv", bufs=1)
            for k in range(D_CONV):
                nc.tensor.matmul(
                    pt[:], convdiag[k][:, c * P:(c + 1) * P],
                    uT[c][:, k + n * 512: k + n * 512 + 512],
                    start=(k == 0), stop=(k == D_CONV - 1),
                )
            nc.scalar.activation(
                ucT[c][:, n * 512:(n + 1) * 512], pt[:], AF.Silu, bias=convb[c][:, 0:1]
            )

    # ---- GEMM2: dblT = xp_w @ uc^T ----
    for n in range(NN):
        pt = psp.tile([64, 512], F32, tag="g2", name="g2", bufs=1)
        for c in range(NC_D):
            nc.tensor.matmul(
                pt[:], xpwT[c][:], ucT[c][:, n * 512:(n + 1) * 512],
                start=(c == 0), stop=(c == NC_D - 1),
            )
        nc.vector.tensor_copy(dt_bf[:, n * 512:(n + 1) * 512], pt[0:DT_RANK, :])
        nc.vector.tensor_copy(bc_bf[:, n * 512:(n + 1) * 512], pt[DT_RANK:64, :])

    # ---- GEMM3 + softplus: delta = ln(1 + exp(dt_w @ dtT + dt_b)) ----
    for m in range(NC_D):
        for n in range(NN):
            pt = psp.tile([P, 512], F32, tag="g3", name="g3", bufs=1)
            nc.tensor.matmul(
                pt[:], dtwT[:, m * P:(m + 1) * P], dt_bf[:, n * 512:(n + 1) * 512],
                start=True, stop=True,
            )
            et = pools["sp"].tile([P, 512], F32, tag="sp_e", name="sp_e", bufs=2)
            nc.scalar.activation(et[:], pt[:], AF.Exp, bias=dtb[m][:, 0:1])
            nc.scalar.activation(
                delta[m][:, n * 512:(n + 1) * 512], et[:], AF.Ln, bias=1.0
            )

    # ---- w = delta * uc ; y init = uc * D ----
    for c in range(NC_D):
        nc.vector.tensor_mul(w_bf[c][:], delta[c][:], ucT[c][:])
        nc.vector.tensor_scalar(y_sb[c][:], ucT[c][:], Dp[c][:, 0:1], None, op0=OP.mult)

    return {"delta": delta, "w": w_bf, "y": y_sb, "bc_bf": bc_bf,
            "A": A_sb, "oht": oht}


def _scan_phase(tc, pools, st, p, d):
    """Selective scan + gate + out-proj for one direction."""
    nc = tc.nc
    psp, bcp, ab, trans = pools["psum"], pools["bc"], pools["ab"], pools["trans"]
    delta, w_bf, y_sb = st["delta"], st["w"], st["y"]
    A_sb, bc_bf, oht = st["A"], st["bc_bf"], st["oht"]

    for s in range(D_STATE):
        Bbc = bcp.tile([P, SEQ], BF16, tag="Bbc", name="Bbc", bufs=2)
        Cbc = bcp.tile([P, SEQ], BF16, tag="Cbc", name="Cbc", bufs=2)
        for src_row, dst in ((s, Bbc), (D_STATE + s, Cbc)):
            for n in range(NN):
                ps = psp.tile([P, 512], F32, tag="bc", name="bcps", bufs=2)
                nc.tensor.matmul(
                    ps[:], oht[:, src_row * P:(src_row + 1) * P],
                    bc_bf[:, n * 512:(n + 1) * 512],
                    start=True, stop=True,
                )
                nc.scalar.copy(dst[:, n * 512:(n + 1) * 512], ps[:])
        for c in range(NC_D):
            a_t = ab.tile([P, SEQ], BF16, tag="a", name="a", bufs=3)
            nc.scalar.activation(a_t[:], delta[c][:], AF.Exp, scale=A_sb[c][:, s:s + 1])
            b_t = ab.tile([P, SEQ], BF16, tag="b", name="b", bufs=3)
            nc.vector.tensor_mul(b_t[:], w_bf[c][:], Bbc[:])
            h_t = ab.tile([P, SEQ], BF16, tag="h", name="h", bufs=2)
            nc.vector.tensor_tensor_scan(
                h_t[:], a_t[:], b_t[:], 0.0, op0=OP.mult, op1=OP.add
            )
            pr = ab.tile([P, SEQ], BF16, tag="pr", name="pr", bufs=2)
            nc.gpsimd.tensor_tensor(pr[:], h_t[:], Cbc[:], op=OP.mult)
            eng = nc.vector if s in YADD_DVE_S else nc.gpsimd
            eng.tensor_tensor(y_sb[c][:], y_sb[c][:], pr[:], op=OP.add)

    # ---- gate: g = y * silu(z) ; GEMM4 ----
    outwT = [trans.tile([P, D_MODEL], BF16, tag="outwT", name=f"outwT{d}{c}", bufs=8) for c in range(NC_D)]
    for c in range(NC_D):
        nc.sync.dma_start(outwT[c][:], p["outwT"][c * P:(c + 1) * P, :])
    g = []
    for c in range(NC_D):
        zin = trans.tile([P, SEQ], BF16, tag="zin", name=f"zin{d}{c}", bufs=2)
        nc.sync.dma_start(zin[:], p["zscr"][c * P:(c + 1) * P, :])
        gt = trans.tile([P, SEQ], BF16, tag="g", name=f"g{d}{c}", bufs=8)
        nc.vector.tensor_mul(gt[:], y_sb[c][:], zin[:])
        g.append(gt)
    for m in range(SEQ // P):
        pt = psp.tile([P, D_MODEL], F32, tag="g4", name="g4", bufs=1)
        for c in range(NC_D):
            nc.tensor.matmul(
                pt[:], g[c][:, m * P:(m + 1) * P], outwT[c][:],
                start=(c == 0), stop=(c == NC_D - 1),
            )
        ot = pools["sp"].tile([P, D_MODEL], F32, tag="ot", name="ot", bufs=2)
        nc.vector.tensor_copy(ot[:], pt[:])
        nc.sync.dma_start(p["out"][m * P:(m + 1) * P, :], ot[:])


def _split_excess_waits(nc):
    """walrus accepts at most one sync-wait per instruction (two for
    EventSemaphore); hoist the excess onto injected same-engine NoOps."""
    for f in nc.m.functions:
        for bb in f.blocks:
            new_insts = []
            for inst in bb.instructions:
                si = inst.sync_info
                cap = 2 if isinstance(inst, mybir.InstEventSemaphore) else 1
                if si is not None and len(si.on_wait) > cap:
                    waits = list(si.on_wait)
                    for i, wv in enumerate(waits[:-cap]):
                        nop = mybir.InstNoOp(name=f"{inst.name}-wsplit{i}", ins=[], outs=[])
                        nop.engine = inst.engine
                        nop.sync_info = bass_rust.SyncInfo(on_wait=[wv], on_update=[])
                        new_insts.append(nop)
                    inst.sync_info = bass_rust.SyncInfo(
                        on_wait=waits[-cap:], on_update=list(si.on_update)
                    )
                new_insts.append(inst)
            try:
                bb.instructions = new_insts
            except Exception:
                bb.instructions.clear()
                bb.instructions.extend(new_insts)


def build_bass():
    nc = bass.Bass()
    params = {d: _dir_params(nc, d) for d in ("f", "b")}
    oht_p = nc.declare_dram_parameter("oht", [2 * D_STATE, 2 * D_STATE * P], BF16, isOutput=False)
    with tile.TileContext(nc) as tc:
        with tc.tile_pool(name="cst", bufs=1) as cst, \
             tc.tile_pool(name="trans", bufs=2) as trans, \
             tc.tile_pool(name="big", bufs=10) as big, \
             tc.tile_pool(name="sp", bufs=2) as sp, \
             tc.tile_pool(name="bc", bufs=2) as bc, \
             tc.tile_pool(name="ab", bufs=2) as ab, \
             tc.tile_pool(name="psum", bufs=2, space="PSUM") as psum:
            pools = {"cst": cst, "trans": trans, "big": big, "sp": sp,
                     "bc": bc, "ab": ab, "psum": psum}
            oht = cst.tile([2 * D_STATE, 2 * D_STATE * P], BF16, tag="oht", name="oht")
            nc.sync.dma_start(oht[:], oht_p[:])
            st_f = _prelude(tc, pools, params["f"], "f", oht)
            _scan_phase(tc, pools, st_f, params["f"], "f")
            st_b = _prelude(tc, pools, params["b"], "b", oht)
            _scan_phase(tc, pools, st_b, params["b"], "b")
    _split_excess_waits(nc)
    return nc


def _prep_dir(w):
    bf = ml_dtypes.bfloat16
    in_w, conv_w, conv_b, xp_w, dt_w, dt_b, A_log, Dp, out_w = w
    cw = np.asarray(conv_w, np.float32)
    convdiag = np.zeros((D_CONV, P, NC_D, P), np.float32)
    for k in range(D_CONV):
        for c in range(NC_D):
            convdiag[k, :, c, :] = np.diag(cw[c * P:(c + 1) * P, k])
    return {
        "inwT": np.ascontiguousarray(in_w.T).astype(bf),
        "xpwT": np.ascontiguousarray(xp_w.T).astype(bf),
        "dtwT": np.ascontiguousarray(dt_w.T).astype(bf),
        "outwT": np.ascontiguousarray(out_w.T).astype(bf),
        "A": np.ascontiguousarray(-np.exp(np.asarray(A_log, np.float64))).astype(np.float32),
        "convdiag": convdiag.reshape(D_CONV * P, D_INNER).astype(bf),
        "convb": np.ascontiguousarray(conv_b).reshape(D_INNER, 1).astype(np.float32),
        "dtb": np.ascontiguousarray(dt_b).reshape(D_INNER, 1).astype(np.float32),
        "Dp": np.ascontiguousarray(Dp).reshape(D_INNER, 1).astype(np.float32),
    }


_CACHED = {}


def kernel(
    x,
    in_w_f, conv_w_f, conv_b_f, xp_w_f, dt_w_f, dt_b_f, A_log_f, D_f, out_w_f,
    in_w_b, conv_w_b, conv_b_b, xp_w_b, dt_w_b, dt_b_b, A_log_b, D_b, out_w_b,
):
    bf = ml_dtypes.bfloat16
    x = np.asarray(x, dtype=np.float32)

    if "nc" not in _CACHED:
        _CACHED["nc"] = build_bass()
    nc = _CACHED["nc"]

    wf = _prep_dir((in_w_f, conv_w_f, conv_b_f, xp_w_f, dt_w_f, dt_b_f,
                    A_log_f, D_f, out_w_f))
    wb = _prep_dir((in_w_b, conv_w_b, conv_b_b, xp_w_b, dt_w_b, dt_b_b,
                    A_log_b, D_b, out_w_b))
    oht = np.kron(np.eye(2 * D_STATE, dtype=np.float32), np.ones((1, P), np.float32)).astype(bf)

    in_maps = []
    for b in range(BATCH):
        m = {"oht": oht}
        for d, wd in (("f", wf), ("b", wb)):
            for k, v in wd.items():
                m[f"{k}_{d}"] = v
        m["xT_f"] = np.ascontiguousarray(x[b].T).astype(bf)
        m["xT_b"] = np.ascontiguousarray(x[b][::-1].T).astype(bf)
        in_maps.append(m)

    res = run_bass_kernel_spmd(nc, in_maps, core_ids=list(range(BATCH)))
    out = np.empty((BATCH, SEQ, D_MODEL), np.float32)
    for b in range(BATCH):
        rb = res.results[b]
        out[b] = rb["out_f"] + rb["out_b"][::-1]
    return out


# revision 6
# speedup vs baseline: 1.6108x; 1.0412x over previous
"""Bidirectional Mamba layer on 8 Trainium2 NeuronCores.

Sharding: data-parallel over batch (8 batches -> 8 cores). Each core runs
both directions (fwd on x, bwd on time-reversed x) for its batch.

v2: engine-rebalanced vs the v1 baseline.
  - depthwise conv on PE (diag-block matmuls, PSUM tap accumulation)
  - all PSUM evacuations + the y-accumulation on the Pool engine
  - softplus as Exp+Ln (same ACT table as the 256 scan exps -> no
    activation-table reloads in the scan phase)
  - y initialized to uc*D in the prelude (drops the gate add, frees ucT)
  - z parked in scratch DRAM, reloaded + silu'd at gate time (SBUF diet)
  - big per-direction arrays (delta/w/y) live in tag-rotated pools so the
    two directions share SBUF; dir-b's prelude overlaps dir-f's scan tail
"""

import sys

sys.path.insert(0, "/opt/trn_rl_repo")

import numpy as np
import ml_dtypes

import concourse.bass as bass
import concourse.mybir as mybir
import bass_rust
from concourse import tile
from concourse.bass_utils import run_bass_kernel_spmd

BF16 = mybir.dt.bfloat16
F32 = mybir.dt.float32
AF = mybir.ActivationFunctionType
OP = mybir.AluOpType

D_MODEL = 512
D_INNER = 1024
D_STATE = 16
D_CONV = 4
DT_RANK = 32
BATCH = 8
SEQ = 1024

P = 128
NC_D = D_INNER // P  # 8 d-chunks
NN = SEQ // 512      # 2 psum-free chunks

# y-accumulate adds routed to DVE for these state indices (rest on Pool)
YADD_DVE_S = frozenset()


def _dir_params(nc, d):
    ps = {
        "inwT": nc.declare_dram_parameter(f"inwT_{d}", [D_MODEL, 2 * D_INNER], BF16, isOutput=False),
        "xpwT": nc.declare_dram_parameter(f"xpwT_{d}", [D_INNER, DT_RANK + 2 * D_STATE], BF16, isOutput=False),
        "dtwT": nc.declare_dram_parameter(f"dtwT_{d}", [DT_RANK, D_INNER], BF16, isOutput=False),
        "outwT": nc.declare_dram_parameter(f"outwT_{d}", [D_INNER, D_MODEL], BF16, isOutput=False),
        "A": nc.declare_dram_parameter(f"A_{d}", [D_INNER, D_STATE], F32, isOutput=False),
        "convdiag": nc.declare_dram_parameter(f"convdiag_{d}", [D_CONV * P, D_INNER], BF16, isOutput=False),
        "convb": nc.declare_dram_parameter(f"convb_{d}", [D_INNER, 1], F32, isOutput=False),
        "dtb": nc.declare_dram_parameter(f"dtb_{d}", [D_INNER, 1], F32, isOutput=False),
        "Dp": nc.declare_dram_parameter(f"Dp_{d}", [D_INNER, 1], F32, isOutput=False),
        "xT": nc.declare_dram_parameter(f"xT_{d}", [D_MODEL, SEQ], BF16, isOutput=False),
        "out": nc.declare_dram_parameter(f"out_{d}", [SEQ, D_MODEL], F32, isOutput=True),
    }
    ps["zscr"] = nc.dram_tensor(f"zscr_{d}", [D_INNER, SEQ], BF16)
    return ps


def _prelude(tc, pools, p, d, oht):
    """GEMM1 + conv + GEMM2 + softplus + w/y init for one direction."""
    nc = tc.nc
    cst, trans, psp, big = pools["cst"], pools["trans"], pools["psum"], pools["big"]

    # ---- weights ----
    inwT = [trans.tile([P, 2 * D_INNER], BF16, tag="inwT", name=f"inwT{d}{k}", bufs=4) for k in range(4)]
    xT = [trans.tile([P, SEQ], BF16, tag="xT", name=f"xT{d}{k}", bufs=4) for k in range(4)]
    for k in range(4):
        nc.sync.dma_start(inwT[k][:], p["inwT"][k * P:(k + 1) * P, :])
        nc.sync.dma_start(xT[k][:], p["xT"][k * P:(k + 1) * P, :])
    convdiag = [trans.tile([P, D_INNER], BF16, tag="cvd", name=f"cvd{d}{k}", bufs=4) for k in range(D_CONV)]
    for k in range(D_CONV):
        nc.sync.dma_start(convdiag[k][:], p["convdiag"][k * P:(k + 1) * P, :])
    xpwT = [cst.tile([P, 64], BF16, tag=f"xpwT{d}{c}", name=f"xpwT{d}{c}") for c in range(NC_D)]
    A_sb = [cst.tile([P, D_STATE], F32, tag=f"A{d}{c}", name=f"A{d}{c}") for c in range(NC_D)]
    convb = [cst.tile([P, 1], F32, tag=f"convb{d}{c}", name=f"convb{d}{c}") for c in range(NC_D)]
    dtb = [cst.tile([P, 1], F32, tag=f"dtb{d}{c}", name=f"dtb{d}{c}") for c in range(NC_D)]
    Dp = [cst.tile([P, 1], F32, tag=f"Dp{d}{c}", name=f"Dp{d}{c}") for c in range(NC_D)]
    for c in range(NC_D):
        sl = slice(c * P, (c + 1) * P)
        nc.sync.dma_start(xpwT[c][:], p["xpwT"][sl, :])
        nc.sync.dma_start(A_sb[c][:], p["A"][sl, :])
        nc.sync.dma_start(convb[c][:], p["convb"][sl, :])
        nc.sync.dma_start(dtb[c][:], p["dtb"][sl, :])
        nc.sync.dma_start(Dp[c][:], p["Dp"][sl, :])
    dtwT = cst.tile([DT_RANK, D_INNER], BF16, tag=f"dtwT{d}", name=f"dtwT{d}")
    nc.sync.dma_start(dtwT[:], p["dtwT"][:])

    # rotated big arrays (shared between directions via tag rotation)
    delta = [big.tile([P, SEQ], BF16, tag="delta", name=f"delta{d}{c}", bufs=10) for c in range(NC_D)]
    w_bf = [big.tile([P, SEQ], BF16, tag="w", name=f"w{d}{c}", bufs=10) for c in range(NC_D)]
    y_sb = [big.tile([P, SEQ], BF16, tag="y", name=f"y{d}{c}", bufs=10) for c in range(NC_D)]
    bc_bf = cst.tile([2 * D_STATE, SEQ], BF16, tag=f"bc_bf{d}", name=f"bc_bf{d}")
    dt_bf = trans.tile([DT_RANK, SEQ], BF16, tag="dt_bf", name=f"dt_bf{d}", bufs=2)

    uT = [trans.tile([P, SEQ + D_CONV - 1], BF16, tag="uT", name=f"uT{d}{c}", bufs=4) for c in range(NC_D)]
    ucT = [trans.tile([P, SEQ], BF16, tag="ucT", name=f"ucT{d}{c}", bufs=8) for c in range(NC_D)]
    for c in range(NC_D):
        nc.vector.memset(uT[c][:, 0:D_CONV - 1], 0.0)

    # ---- GEMM1: uzT = in_w @ x^T ----
    for m in range(2 * NC_D):
        for n in range(NN):
            pt = psp.tile([P, 512], F32, tag="g1", name="g1", bufs=2)
            for k in range(4):
                nc.tensor.matmul(
                    pt[:], inwT[k][:, m * P:(m + 1) * P],
                    xT[k][:, n * 512:(n + 1) * 512],
                    start=(k == 0), stop=(k == 3),
                )
            if m < NC_D:
                nc.scalar.copy(
                    uT[m][:, D_CONV - 1 + n * 512: D_CONV - 1 + (n + 1) * 512], pt[:]
                )
            else:
                zst = pools["sp"].tile([P, 512], BF16, tag="zst", name="zst", bufs=3)
                nc.scalar.activation(zst[:], pt[:], AF.Silu)
                nc.sync.dma_start(
                    p["zscr"][(m - NC_D) * P:(m - NC_D + 1) * P, n * 512:(n + 1) * 512],
                    zst[:],
                )

    # ---- conv on PE: uc = silu(sum_k diag(w_k) @ u_shift_k + b) ----
    for c in range(NC_D):
        for n in range(NN):
            pt = psp.tile([P, 512], F32, tag="cv", name="cv", bufs=1)
            for k in range(D_CONV):
                nc.tensor.matmul(
                    pt[:], convdiag[k][:, c * P:(c + 1) * P],
                    uT[c][:, k + n * 512: k + n * 512 + 512],
                    start=(k == 0), stop=(k == D_CONV - 1),
                )
            nc.scalar.activation(
                ucT[c][:, n * 512:(n + 1) * 512], pt[:], AF.Silu, bias=convb[c][:, 0:1]
            )

    # ---- GEMM2: dblT = xp_w @ uc^T ----
    for n in range(NN):
        pt = psp.tile([64, 512], F32, tag="g2", name="g2", bufs=1)
        for c in range(NC_D):
            nc.tensor.matmul(
                pt[:], xpwT[c][:], ucT[c][:, n * 512:(n + 1) * 512],
                start=(c == 0), stop=(c == NC_D - 1),
            )
        nc.vector.tensor_copy(dt_bf[:, n * 512:(n + 1) * 512], pt[0:DT_RANK, :])
        nc.vector.tensor_copy(bc_bf[:, n * 512:(n + 1) * 512], pt[DT_RANK:64, :])

    # ---- GEMM3 + softplus: delta = ln(1 + exp(dt_w @ dtT + dt_b)) ----
    for m in range(NC_D):
        for n in range(NN):
            pt = psp.tile([P, 512], F32, tag="g3", name="g3", bufs=1)
            nc.tensor.matmul(
                pt[:], dtwT[:, m * P:(m + 1) * P], dt_bf[:, n * 512:(n + 1) * 512],
                start=True, stop=True,
            )
            et = pools["sp"].tile([P, 512], F32, tag="sp_e", name="sp_e", bufs=2)
            nc.scalar.activation(et[:], pt[:], AF.Exp, bias=dtb[m][:, 0:1])
            nc.scalar.activation(
                delta[m][:, n * 512:(n + 1) * 512], et[:], AF.Ln, bias=1.0
            )

    # ---- w = delta * uc ; y init = uc * D ----
    for c in range(NC_D):
        nc.gpsimd.tensor_tensor(w_bf[c][:], delta[c][:], ucT[c][:], op=OP.mult)
        nc.vector.tensor_scalar(y_sb[c][:], ucT[c][:], Dp[c][:, 0:1], None, op0=OP.mult)

    return {"delta": delta, "w": w_bf, "y": y_sb, "bc_bf": bc_bf,
            "A": A_sb, "oht": oht}


def _scan_phase(tc, pools, st, p, d):
    """Selective scan + gate + out-proj for one direction."""
    nc = tc.nc
    psp, bcp, ab, trans = pools["psum"], pools["bc"], pools["ab"], pools["trans"]
    delta, w_bf, y_sb = st["delta"], st["w"], st["y"]
    A_sb, bc_bf, oht = st["A"], st["bc_bf"], st["oht"]

    for s in range(D_STATE):
        Bbc = bcp.tile([P, SEQ], BF16, tag="Bbc", name="Bbc", bufs=2)
        Cbc = bcp.tile([P, SEQ], BF16, tag="Cbc", name="Cbc", bufs=2)
        for src_row, dst in ((s, Bbc), (D_STATE + s, Cbc)):
            for n in range(NN):
                ps = psp.tile([P, 512], F32, tag="bc", name="bcps", bufs=2)
                nc.tensor.matmul(
                    ps[:], oht[:, src_row * P:(src_row + 1) * P],
                    bc_bf[:, n * 512:(n + 1) * 512],
                    start=True, stop=True,
                )
                nc.scalar.copy(dst[:, n * 512:(n + 1) * 512], ps[:])
        for c in range(NC_D):
            a_t = ab.tile([P, SEQ], BF16, tag="a", name="a", bufs=3)
            nc.scalar.activation(a_t[:], delta[c][:], AF.Exp, scale=A_sb[c][:, s:s + 1])
            b_t = ab.tile([P, SEQ], BF16, tag="b", name="b", bufs=3)
            nc.vector.tensor_mul(b_t[:], w_bf[c][:], Bbc[:])
            h_t = ab.tile([P, SEQ], BF16, tag="h", name="h", bufs=2)
            nc.vector.tensor_tensor_scan(
                h_t[:], a_t[:], b_t[:], 0.0, op0=OP.mult, op1=OP.add
            )
            pr = ab.tile([P, SEQ], BF16, tag="pr", name="pr", bufs=2)
            nc.gpsimd.tensor_tensor(pr[:], h_t[:], Cbc[:], op=OP.mult)
            eng = nc.vector if s in YADD_DVE_S else nc.gpsimd
            eng.tensor_tensor(y_sb[c][:], y_sb[c][:], pr[:], op=OP.add)

    # ---- gate: g = y * silu(z) ; GEMM4 ----
    outwT = [trans.tile([P, D_MODEL], BF16, tag="outwT", name=f"outwT{d}{c}", bufs=8) for c in range(NC_D)]
    for c in range(NC_D):
        nc.sync.dma_start(outwT[c][:], p["outwT"][c * P:(c + 1) * P, :])
    g = []
    for c in range(NC_D):
        zin = trans.tile([P, SEQ], BF16, tag="zin", name=f"zin{d}{c}", bufs=2)
        nc.sync.dma_start(zin[:], p["zscr"][c * P:(c + 1) * P, :])
        gt = trans.tile([P, SEQ], BF16, tag="g", name=f"g{d}{c}", bufs=8)
        nc.gpsimd.tensor_tensor(gt[:], y_sb[c][:], zin[:], op=OP.mult)
        g.append(gt)
    for m in range(SEQ // P):
        pt = psp.tile([P, D_MODEL], F32, tag="g4", name="g4", bufs=1)
        for c in range(NC_D):
            nc.tensor.matmul(
                pt[:], g[c][:, m * P:(m + 1) * P], outwT[c][:],
                start=(c == 0), stop=(c == NC_D - 1),
            )
        ot = pools["sp"].tile([P, D_MODEL], F32, tag="ot", name="ot", bufs=2)
        nc.vector.tensor_copy(ot[:], pt[:])
        nc.sync.dma_start(p["out"][m * P:(m + 1) * P, :], ot[:])


def _split_excess_waits(nc):
    """walrus accepts at most one sync-wait per instruction (two for
    EventSemaphore); hoist the excess onto injected same-engine NoOps."""
    for f in nc.m.functions:
        for bb in f.blocks:
            new_insts = []
            for inst in bb.instructions:
                si = inst.sync_info
                cap = 2 if isinstance(inst, mybir.InstEventSemaphore) else 1
                if si is not None and len(si.on_wait) > cap:
                    waits = list(si.on_wait)
                    for i, wv in enumerate(waits[:-cap]):
                        nop = mybir.InstNoOp(name=f"{inst.name}-wsplit{i}", ins=[], outs=[])
                        nop.engine = inst.engine
                        nop.sync_info = bass_rust.SyncInfo(on_wait=[wv], on_update=[])
                        new_insts.append(nop)
                    inst.sync_info = bass_rust.SyncInfo(
                        on_wait=waits[-cap:], on_update=list(si.on_update)
                    )
                new_insts.append(inst)
            try:
                bb.instructions = new_insts
            except Exception:
                bb.instructions.clear()
                bb.instructions.extend(new_insts)


def build_bass():
    nc = bass.Bass()
    params = {d: _dir_params(nc, d) for d in ("f", "b")}
    oht_p = nc.declare_dram_parameter("oht", [2 * D_STATE, 2 * D_STATE * P], BF16, isOutput=False)
    with tile.TileContext(nc) as tc:
        with tc.tile_pool(name="cst", bufs=1) as cst, \
             tc.tile_pool(name="trans", bufs=2) as trans, \
             tc.tile_pool(name="big", bufs=10) as big, \
             tc.tile_pool(name="sp", bufs=2) as sp, \
             tc.tile_pool(name="bc", bufs=2) as bc, \
             tc.tile_pool(name="ab", bufs=2) as ab, \
             tc.tile_pool(name="psum", bufs=2, space="PSUM") as psum:
            pools = {"cst": cst, "trans": trans, "big": big, "sp": sp,
                     "bc": bc, "ab": ab, "psum": psum}
            oht = cst.tile([2 * D_STATE, 2 * D_STATE * P], BF16, tag="oht", name="oht")
            nc.sync.dma_start(oht[:], oht_p[:])
            st_f = _prelude(tc, pools, params["f"], "f", oht)
            _scan_phase(tc, pools, st_f, params["f"], "f")
            st_b = _prelude(tc, pools, params["b"], "b", oht)
            _scan_phase(tc, pools, st_b, params["b"], "b")
    _split_excess_waits(nc)
    return nc


def _prep_dir(w):
    bf = ml_dtypes.bfloat16
    in_w, conv_w, conv_b, xp_w, dt_w, dt_b, A_log, Dp, out_w = w
    cw = np.asarray(conv_w, np.float32)
    convdiag = np.zeros((D_CONV, P, NC_D, P), np.float32)
    for k in range(D_CONV):
        for c in range(NC_D):
            convdiag[k, :, c, :] = np.diag(cw[c * P:(c + 1) * P, k])
    return {
        "inwT": np.ascontiguousarray(in_w.T).astype(bf),
        "xpwT": np.ascontiguousarray(xp_w.T).astype(bf),
        "dtwT": np.ascontiguousarray(dt_w.T).astype(bf),
        "outwT": np.ascontiguousarray(out_w.T).astype(bf),
        "A": np.ascontiguousarray(-np.exp(np.asarray(A_log, np.float64))).astype(np.float32),
        "convdiag": convdiag.reshape(D_CONV * P, D_INNER).astype(bf),
        "convb": np.ascontiguousarray(conv_b).reshape(D_INNER, 1).astype(np.float32),
        "dtb": np.ascontiguousarray(dt_b).reshape(D_INNER, 1).astype(np.float32),
        "Dp": np.ascontiguousarray(Dp).reshape(D_INNER, 1).astype(np.float32),
    }


_CACHED = {}


def kernel(
    x,
    in_w_f, conv_w_f, conv_b_f, xp_w_f, dt_w_f, dt_b_f, A_log_f, D_f, out_w_f,
    in_w_b, conv_w_b, conv_b_b, xp_w_b, dt_w_b, dt_b_b, A_log_b, D_b, out_w_b,
):
    bf = ml_dtypes.bfloat16
    x = np.asarray(x, dtype=np.float32)

    if "nc" not in _CACHED:
        _CACHED["nc"] = build_bass()
    nc = _CACHED["nc"]

    wf = _prep_dir((in_w_f, conv_w_f, conv_b_f, xp_w_f, dt_w_f, dt_b_f,
                    A_log_f, D_f, out_w_f))
    wb = _prep_dir((in_w_b, conv_w_b, conv_b_b, xp_w_b, dt_w_b, dt_b_b,
                    A_log_b, D_b, out_w_b))
    oht = np.kron(np.eye(2 * D_STATE, dtype=np.float32), np.ones((1, P), np.float32)).astype(bf)

    in_maps = []
    for b in range(BATCH):
        m = {"oht": oht}
        for d, wd in (("f", wf), ("b", wb)):
            for k, v in wd.items():
                m[f"{k}_{d}"] = v
        m["xT_f"] = np.ascontiguousarray(x[b].T).astype(bf)
        m["xT_b"] = np.ascontiguousarray(x[b][::-1].T).astype(bf)
        in_maps.append(m)

    res = run_bass_kernel_spmd(nc, in_maps, core_ids=list(range(BATCH)))
    out = np.empty((BATCH, SEQ, D_MODEL), np.float32)
    for b in range(BATCH):
        rb = res.results[b]
        out[b] = rb["out_f"] + rb["out_b"][::-1]
    return out


# revision 8
# speedup vs baseline: 1.7006x; 1.0557x over previous
"""Bidirectional Mamba layer on 8 Trainium2 NeuronCores.

Sharding: data-parallel over batch (8 batches -> 8 cores). Each core runs
both directions (fwd on x, bwd on time-reversed x) for its batch.

v3: engine-rebalanced + software-pipelined across directions.
  - depthwise conv on PE (diag-block matmuls, PSUM tap accumulation)
  - y-mul/y-add/w-mul/gate-mul on the Pool engine; PSUM evacuations on
    ACT (Copy is in every ACT table -> no table reloads); scans + b-mul
    + tensor_scalar on DVE
  - softplus as Exp+Ln (same ACT table as the 256 scan exps)
  - y initialized to uc*D in the prelude (drops the gate add, frees ucT)
  - z parked in scratch DRAM (f: pre-silu'd; b: raw, silu at gate)
  - dir-b's GEMM1+conv are emitted interleaved into dir-f's scan loop
    (engines execute in program order, so emission order is the schedule);
    b's conv nonlinearity uses the tanh identity silu(x)=x*(1+tanh(x/2))/2
    so it shares an ACT table with the concurrent scan exps
  - big per-direction arrays (delta/w/y) live in tag-rotated pools
"""

import sys

sys.path.insert(0, "/opt/trn_rl_repo")

import numpy as np
import ml_dtypes

import concourse.bass as bass
import concourse.mybir as mybir
import bass_rust
from concourse import tile
from concourse.bass_utils import run_bass_kernel_spmd

BF16 = mybir.dt.bfloat16
F32 = mybir.dt.float32
AF = mybir.ActivationFunctionType
OP = mybir.AluOpType

D_MODEL = 512
D_INNER = 1024
D_STATE = 16
D_CONV = 4
DT_RANK = 32
BATCH = 8
SEQ = 1024

P = 128
NC_D = D_INNER // P  # 8 d-chunks
NN = SEQ // 512      # 2 psum-free chunks


def _dir_params(nc, d):
    ps = {
        "inwT": nc.declare_dram_parameter(f"inwT_{d}", [D_MODEL, 2 * D_INNER], BF16, isOutput=False),
        "xpwT": nc.declare_dram_parameter(f"xpwT_{d}", [D_INNER, DT_RANK + 2 * D_STATE], BF16, isOutput=False),
        "dtwT": nc.declare_dram_parameter(f"dtwT_{d}", [DT_RANK, D_INNER], BF16, isOutput=False),
        "outwT": nc.declare_dram_parameter(f"outwT_{d}", [D_INNER, D_MODEL], BF16, isOutput=False),
        "A": nc.declare_dram_parameter(f"A_{d}", [D_INNER, D_STATE], F32, isOutput=False),
        "convdiag": nc.declare_dram_parameter(f"convdiag_{d}", [D_CONV * P, D_INNER], BF16, isOutput=False),
        "convb": nc.declare_dram_parameter(f"convb_{d}", [D_INNER, 1], F32, isOutput=False),
        "convbh": nc.declare_dram_parameter(f"convbh_{d}", [D_INNER, 1], F32, isOutput=False),
        "dtb": nc.declare_dram_parameter(f"dtb_{d}", [D_INNER, 1], F32, isOutput=False),
        "Dp": nc.declare_dram_parameter(f"Dp_{d}", [D_INNER, 1], F32, isOutput=False),
        "xT": nc.declare_dram_parameter(f"xT_{d}", [D_MODEL, SEQ], BF16, isOutput=False),
        "out": nc.declare_dram_parameter(f"out_{d}", [SEQ, D_MODEL], F32, isOutput=True),
    }
    ps["zscr"] = nc.dram_tensor(f"zscr_{d}", [D_INNER, SEQ], BF16)
    return ps


def _load_weights(tc, pools, p, d):
    nc = tc.nc
    cst, trans = pools["cst"], pools["trans"]
    st = {}
    st["inwT"] = [trans.tile([P, 2 * D_INNER], BF16, tag="inwT", name=f"inwT{d}{k}", bufs=4) for k in range(4)]
    st["xT"] = [trans.tile([P, SEQ], BF16, tag="xT", name=f"xT{d}{k}", bufs=4) for k in range(4)]
    for k in range(4):
        nc.sync.dma_start(st["inwT"][k][:], p["inwT"][k * P:(k + 1) * P, :])
        nc.sync.dma_start(st["xT"][k][:], p["xT"][k * P:(k + 1) * P, :])
    st["convdiag"] = [trans.tile([P, D_INNER], BF16, tag="cvd", name=f"cvd{d}{k}", bufs=4) for k in range(D_CONV)]
    for k in range(D_CONV):
        nc.sync.dma_start(st["convdiag"][k][:], p["convdiag"][k * P:(k + 1) * P, :])
    for nm, width, dt in (("xpwT", 64, BF16), ("A", D_STATE, F32), ("convb", 1, F32),
                          ("convbh", 1, F32), ("dtb", 1, F32), ("Dp", 1, F32)):
        st[nm] = [cst.tile([P, width], dt, tag=f"{nm}{d}{c}", name=f"{nm}{d}{c}") for c in range(NC_D)]
        for c in range(NC_D):
            nc.sync.dma_start(st[nm][c][:], p[nm][c * P:(c + 1) * P, :])
    st["dtwT"] = cst.tile([DT_RANK, D_INNER], BF16, tag=f"dtwT{d}", name=f"dtwT{d}")
    nc.sync.dma_start(st["dtwT"][:], p["dtwT"][:])

    st["delta"] = [pools["big"].tile([P, SEQ], BF16, tag="delta", name=f"delta{d}{c}", bufs=10) for c in range(NC_D)]
    st["w"] = [pools["big"].tile([P, SEQ], BF16, tag="w", name=f"w{d}{c}", bufs=10) for c in range(NC_D)]
    st["bc_bf"] = cst.tile([2 * D_STATE, SEQ], BF16, tag=f"bc_bf{d}", name=f"bc_bf{d}")
    st["dt_bf"] = trans.tile([DT_RANK, SEQ], BF16, tag="dt_bf", name=f"dt_bf{d}", bufs=1)
    st["uT"] = [trans.tile([P, SEQ + D_CONV - 1], BF16, tag="uT", name=f"uT{d}{c}", bufs=4) for c in range(NC_D)]
    st["ucT"] = [trans.tile([P, SEQ], BF16, tag="ucT", name=f"ucT{d}{c}", bufs=8) for c in range(NC_D)]
    for c in range(NC_D):
        nc.vector.memset(st["uT"][c][:, 0:D_CONV - 1], 0.0)
    return st


def _gemm1_conv_units(tc, pools, p, d, st, overlap):
    """Yield after each GEMM1 (m,n) unit and each conv (c,n) unit.

    overlap=False: conv nonlinearity is a direct ACT Silu; z is silu'd at
    staging time. overlap=True (emitted amid the other direction's scan
    exps): conv uses the tanh identity, z is staged raw.
    """
    nc = tc.nc
    psp, sp = pools["psum"], pools["sp"]
    inwT, xT, uT, ucT = st["inwT"], st["xT"], st["uT"], st["ucT"]

    def g1_unit(m, n):
        pt = psp.tile([P, 512], F32, tag="g1", name="g1", bufs=2)
        for k in range(4):
            nc.tensor.matmul(
                pt[:], inwT[k][:, m * P:(m + 1) * P],
                xT[k][:, n * 512:(n + 1) * 512],
                start=(k == 0), stop=(k == 3),
            )
        if m < NC_D:
            nc.scalar.copy(
                uT[m][:, D_CONV - 1 + n * 512: D_CONV - 1 + (n + 1) * 512], pt[:]
            )
        else:
            zst = sp.tile([P, 512], BF16, tag="zst", name="zst", bufs=2)
            if overlap:
                nc.scalar.copy(zst[:], pt[:])
            else:
                nc.scalar.activation(zst[:], pt[:], AF.Silu)
            nc.sync.dma_start(
                p["zscr"][(m - NC_D) * P:(m - NC_D + 1) * P, n * 512:(n + 1) * 512],
                zst[:],
            )

    def cv_unit(c, n):
        pt = psp.tile([P, 512], F32, tag="cv", name="cv", bufs=1)
        for k in range(D_CONV):
            nc.tensor.matmul(
                pt[:], st["convdiag"][k][:, c * P:(c + 1) * P],
                uT[c][:, k + n * 512: k + n * 512 + 512],
                start=(k == 0), stop=(k == D_CONV - 1),
            )
        sl = slice(n * 512, (n + 1) * 512)
        if not overlap:
            nc.scalar.activation(ucT[c][:, sl], pt[:], AF.Silu, bias=st["convb"][c][:, 0:1])
        else:
            # silu(x) = x*(1+tanh(x/2))/2; ch = x/2 (+convb/2 bias), th = tanh(x/2)
            ch = sp.tile([P, 512], BF16, tag="ch", name="ch", bufs=2)
            nc.scalar.activation(ch[:], pt[:], AF.Identity, bias=st["convbh"][c][:, 0:1], scale=0.5)
            th = sp.tile([P, 512], BF16, tag="th", name="th", bufs=2)
            nc.scalar.activation(th[:], pt[:], AF.Tanh, bias=st["convbh"][c][:, 0:1], scale=0.5)
            t1 = sp.tile([P, 512], BF16, tag="t1", name="t1", bufs=2)
            nc.gpsimd.tensor_scalar(t1[:], th[:], 1.0, None, op0=OP.add)
            nc.gpsimd.tensor_tensor(ucT[c][:, sl], t1[:], ch[:], op=OP.mult)

    # u-part GEMM1 with conv chasing one chunk behind (keeps uT rotation shallow)
    for m in range(NC_D):
        for n in range(NN):
            g1_unit(m, n)
            yield
        if m >= 1:
            for n in range(NN):
                cv_unit(m - 1, n)
                yield
    for n in range(NN):
        cv_unit(NC_D - 1, n)
        yield
    # z-part GEMM1
    for m in range(NC_D, 2 * NC_D):
        for n in range(NN):
            g1_unit(m, n)
            yield


def _prelude_tail(tc, pools, p, d, st):
    """GEMM2 + GEMM3/softplus + w-mul + y-init (after conv is done)."""
    nc = tc.nc
    psp = pools["psum"]
    ucT, dt_bf, bc_bf = st["ucT"], st["dt_bf"], st["bc_bf"]

    for n in range(NN):
        pt = psp.tile([64, 512], F32, tag="g2", name="g2", bufs=1)
        for c in range(NC_D):
            nc.tensor.matmul(
                pt[:], st["xpwT"][c][:], ucT[c][:, n * 512:(n + 1) * 512],
                start=(c == 0), stop=(c == NC_D - 1),
            )
        nc.vector.tensor_copy(dt_bf[:, n * 512:(n + 1) * 512], pt[0:DT_RANK, :])
        nc.vector.tensor_copy(bc_bf[:, n * 512:(n + 1) * 512], pt[DT_RANK:64, :])

    delta = st["delta"]
    for m in range(NC_D):
        for n in range(NN):
            pt = psp.tile([P, 512], F32, tag="g3", name="g3", bufs=1)
            nc.tensor.matmul(
                pt[:], st["dtwT"][:, m * P:(m + 1) * P], dt_bf[:, n * 512:(n + 1) * 512],
                start=True, stop=True,
            )
            et = pools["sp"].tile([P, 512], F32, tag="sp_e", name="sp_e", bufs=2)
            nc.scalar.activation(et[:], pt[:], AF.Exp, bias=st["dtb"][m][:, 0:1])
            nc.scalar.activation(delta[m][:, n * 512:(n + 1) * 512], et[:], AF.Ln, bias=1.0)

    st["y"] = [pools["big"].tile([P, SEQ], BF16, tag="y", name=f"y{d}{c}", bufs=10) for c in range(NC_D)]
    for c in range(NC_D):
        nc.gpsimd.tensor_tensor(st["w"][c][:], delta[c][:], ucT[c][:], op=OP.mult)
        nc.vector.tensor_scalar(st["y"][c][:], ucT[c][:], st["Dp"][c][:, 0:1], None, op0=OP.mult)


def _scan_iter(tc, pools, st, oht, s):
    """One state-index iteration of the selective scan."""
    nc = tc.nc
    psp, bcp, ab = pools["psum"], pools["bc"], pools["ab"]
    delta, w_bf, y_sb, A_sb, bc_bf = st["delta"], st["w"], st["y"], st["A"], st["bc_bf"]

    Bbc = bcp.tile([P, SEQ], BF16, tag="Bbc", name="Bbc", bufs=2)
    Cbc = bcp.tile([P, SEQ], BF16, tag="Cbc", name="Cbc", bufs=2)
    for src_row, dst in ((s, Bbc), (D_STATE + s, Cbc)):
        for n in range(NN):
            ps = psp.tile([P, 512], F32, tag="bc", name="bcps", bufs=2)
            nc.tensor.matmul(
                ps[:], oht[:, src_row * P:(src_row + 1) * P],
                bc_bf[:, n * 512:(n + 1) * 512],
                start=True, stop=True,
            )
            nc.scalar.copy(dst[:, n * 512:(n + 1) * 512], ps[:])
    for c in range(NC_D):
        a_t = ab.tile([P, SEQ], BF16, tag="a", name="a", bufs=2)
        nc.scalar.activation(a_t[:], delta[c][:], AF.Exp, scale=A_sb[c][:, s:s + 1])
        b_t = ab.tile([P, SEQ], BF16, tag="b", name="b", bufs=2)
        nc.vector.tensor_mul(b_t[:], w_bf[c][:], Bbc[:])
        h_t = ab.tile([P, SEQ], BF16, tag="h", name="h", bufs=2)
        nc.vector.tensor_tensor_scan(
            h_t[:], a_t[:], b_t[:], 0.0, op0=OP.mult, op1=OP.add
        )
        pr = ab.tile([P, SEQ], BF16, tag="pr", name="pr", bufs=2)
        nc.gpsimd.tensor_tensor(pr[:], h_t[:], Cbc[:], op=OP.mult)
        nc.gpsimd.tensor_tensor(y_sb[c][:], y_sb[c][:], pr[:], op=OP.add)


def _gate_gemm4(tc, pools, st, p, d, z_needs_silu):
    nc = tc.nc
    psp, trans = pools["psum"], pools["trans"]
    y_sb = st["y"]

    outwT = [trans.tile([P, D_MODEL], BF16, tag="outwT", name=f"outwT{d}{c}", bufs=8) for c in range(NC_D)]
    for c in range(NC_D):
        nc.sync.dma_start(outwT[c][:], p["outwT"][c * P:(c + 1) * P, :])
    g = []
    for c in range(NC_D):
        zin = trans.tile([P, SEQ], BF16, tag="zin", name=f"zin{d}{c}", bufs=2)
        nc.sync.dma_start(zin[:], p["zscr"][c * P:(c + 1) * P, :])
        if z_needs_silu:
            nc.scalar.activation(zin[:], zin[:], AF.Silu)
        gt = trans.tile([P, SEQ], BF16, tag="g", name=f"g{d}{c}", bufs=8)
        nc.gpsimd.tensor_tensor(gt[:], y_sb[c][:], zin[:], op=OP.mult)
        g.append(gt)
    for m in range(SEQ // P):
        pt = psp.tile([P, D_MODEL], F32, tag="g4", name="g4", bufs=1)
        for c in range(NC_D):
            nc.tensor.matmul(
                pt[:], g[c][:, m * P:(m + 1) * P], outwT[c][:],
                start=(c == 0), stop=(c == NC_D - 1),
            )
        ot = pools["sp"].tile([P, D_MODEL], F32, tag="ot", name="ot", bufs=2)
        nc.vector.tensor_copy(ot[:], pt[:])
        nc.sync.dma_start(p["out"][m * P:(m + 1) * P, :], ot[:])


def _split_excess_waits(nc):
    """walrus accepts at most one sync-wait per instruction (two for
    EventSemaphore); hoist the excess onto injected same-engine NoOps."""
    for f in nc.m.functions:
        for bb in f.blocks:
            new_insts = []
            for inst in bb.instructions:
                si = inst.sync_info
                cap = 2 if isinstance(inst, mybir.InstEventSemaphore) else 1
                if si is not None and len(si.on_wait) > cap:
                    waits = list(si.on_wait)
                    for i, wv in enumerate(waits[:-cap]):
                        nop = mybir.InstNoOp(name=f"{inst.name}-wsplit{i}", ins=[], outs=[])
                        nop.engine = inst.engine
                        nop.sync_info = bass_rust.SyncInfo(on_wait=[wv], on_update=[])
                        new_insts.append(nop)
                    inst.sync_info = bass_rust.SyncInfo(
                        on_wait=waits[-cap:], on_update=list(si.on_update)
                    )
                new_insts.append(inst)
            try:
                bb.instructions = new_insts
            except Exception:
                bb.instructions.clear()
                bb.instructions.extend(new_insts)


def build_bass():
    nc = bass.Bass()
    params = {d: _dir_params(nc, d) for d in ("f", "b")}
    oht_p = nc.declare_dram_parameter("oht", [2 * D_STATE, 2 * D_STATE * P], BF16, isOutput=False)
    with tile.TileContext(nc) as tc:
        with tc.tile_pool(name="cst", bufs=1) as cst, \
             tc.tile_pool(name="trans", bufs=2) as trans, \
             tc.tile_pool(name="big", bufs=10) as big, \
             tc.tile_pool(name="sp", bufs=2) as sp, \
             tc.tile_pool(name="bc", bufs=2) as bc, \
             tc.tile_pool(name="ab", bufs=2) as ab, \
             tc.tile_pool(name="psum", bufs=2, space="PSUM") as psum:
            pools = {"cst": cst, "trans": trans, "big": big, "sp": sp,
                     "bc": bc, "ab": ab, "psum": psum}
            oht = cst.tile([2 * D_STATE, 2 * D_STATE * P], BF16, tag="oht", name="oht")
            nc.sync.dma_start(oht[:], oht_p[:])

            # dir f prelude, plain
            st_f = _load_weights(tc, pools, params["f"], "f")
            for _ in _gemm1_conv_units(tc, pools, params["f"], "f", st_f, overlap=False):
                pass
            _prelude_tail(tc, pools, params["f"], "f", st_f)

            # dir f scan, with dir b's GEMM1+conv pumped between iterations
            st_b = _load_weights(tc, pools, params["b"], "b")
            gen_b = _gemm1_conv_units(tc, pools, params["b"], "b", st_b, overlap=True)
            for s in range(D_STATE):
                _scan_iter(tc, pools, st_f, oht, s)
                for _ in range(3):
                    if next(gen_b, StopIteration) is StopIteration:
                        break
            for _ in gen_b:
                pass

            _gate_gemm4(tc, pools, st_f, params["f"], "f", z_needs_silu=False)
            _prelude_tail(tc, pools, params["b"], "b", st_b)
            for s in range(D_STATE):
                _scan_iter(tc, pools, st_b, oht, s)
            _gate_gemm4(tc, pools, st_b, params["b"], "b", z_needs_silu=True)
    _split_excess_waits(nc)
    return nc


def _prep_dir(w):
    bf = ml_dtypes.bfloat16
    in_w, conv_w, conv_b, xp_w, dt_w, dt_b, A_log, Dp, out_w = w
    cw = np.asarray(conv_w, np.float32)
    convdiag = np.zeros((D_CONV, P, NC_D, P), np.float32)
    for k in range(D_CONV):
        for c in range(NC_D):
            convdiag[k, :, c, :] = np.diag(cw[c * P:(c + 1) * P, k])
    cb = np.ascontiguousarray(conv_b).reshape(D_INNER, 1).astype(np.float32)
    return {
        "inwT": np.ascontiguousarray(in_w.T).astype(bf),
        "xpwT": np.ascontiguousarray(xp_w.T).astype(bf),
        "dtwT": np.ascontiguousarray(dt_w.T).astype(bf),
        "outwT": np.ascontiguousarray(out_w.T).astype(bf),
        "A": np.ascontiguousarray(-np.exp(np.asarray(A_log, np.float64))).astype(np.float32),
        "convdiag": convdiag.reshape(D_CONV * P, D_INNER).astype(bf),
        "convb": cb,
        "convbh": cb * 0.5,
        "dtb": np.ascontiguousarray(dt_b).reshape(D_INNER, 1).astype(np.float32),
        "Dp": np.ascontiguousarray(Dp).reshape(D_INNER, 1).astype(np.float32),
    }


_CACHED = {}


def kernel(
    x,
    in_w_f, conv_w_f, conv_b_f, xp_w_f, dt_w_f, dt_b_f, A_log_f, D_f, out_w_f,
    in_w_b, conv_w_b, conv_b_b, xp_w_b, dt_w_b, dt_b_b, A_log_b, D_b, out_w_b,
):
    bf = ml_dtypes.bfloat16
    x = np.asarray(x, dtype=np.float32)

    if "nc" not in _CACHED:
        _CACHED["nc"] = build_bass()
    nc = _CACHED["nc"]

    wf = _prep_dir((in_w_f, conv_w_f, conv_b_f, xp_w_f, dt_w_f, dt_b_f,
                    A_log_f, D_f, out_w_f))
    wb = _prep_dir((in_w_b, conv_w_b, conv_b_b, xp_w_b, dt_w_b, dt_b_b,
                    A_log_b, D_b, out_w_b))
    oht = np.kron(np.eye(2 * D_STATE, dtype=np.float32), np.ones((1, P), np.float32)).astype(bf)

    in_maps = []
    for b in range(BATCH):
        m = {"oht": oht}
        for d, wd in (("f", wf), ("b", wb)):
            for k, v in wd.items():
                m[f"{k}_{d}"] = v
        m["xT_f"] = np.ascontiguousarray(x[b].T).astype(bf)
        m["xT_b"] = np.ascontiguousarray(x[b][::-1].T).astype(bf)
        in_maps.append(m)

    res = run_bass_kernel_spmd(nc, in_maps, core_ids=list(range(BATCH)))
    out = np.empty((BATCH, SEQ, D_MODEL), np.float32)
    for b in range(BATCH):
        rb = res.results[b]
        out[b] = rb["out_f"] + rb["out_b"][::-1]
    return out
